# revision 1
# baseline (speedup 1.0000x reference)
"""Trainium2 Bass kernel: LSTM encoder (scan LSTMCell over T, return final carry).

B=64, T=1024, F=H=512.  Data-parallel over batch: cores 0 and 1 each run the
full recurrence for a 32-row batch half (remaining cores run redundant work
that is discarded).

Per core, per step: gates = [h;x_t;1] @ [Wh;Wx;b] computed as one fused
accumulation in PSUM using 4-way column-tiled bf16 matmuls (tile_position),
with the 4H dimension permuted into 4 "bands" so band q holds gates i,g,f,o
of hidden quarter q on psum partitions [32q:32q+32).

The pointwise chain uses the identity sigmoid(x) = (tanh(x/2)+1)/2 for the
i and o gates, with the 1/2 pre-scales folded into the packed weights and a
doubled cell state C=2c, so the cell update needs only fused ops:
  th_ig = tanh(gates_ig'); sf = sigmoid(gates_f)
  t1 = (th_i+1)*th_g   (scalar-tensor-tensor, DVE)  [= 2 sig_i tanh_g]
  u  = sf*C            (GPSIMD)
  C  = u + t1          (DVE)                        [= 2 c_new]
  tc = tanh(C*0.5)     (ACT, fused scale)           [= tanh(c_new)]
  ho = (th_o+1)*tc     (GPSIMD add + DVE mul)       [= 2 h_new]
h is re-transposed for the next step's stationary operand with 4 PE
transposes (one per row-group, each into its own PSUM bank) + 4 copies.
x_t^T tiles are staged 8 steps at a time with one DMA-xbar transpose per
chunk, prefetched one loop body ahead (including the next body's step 0).
An int32 input r repeats the whole scan (r=1 computes the real output; r>1
is used for differential on-device timing).
"""
import sys
sys.path.insert(0, '/opt/trn_rl_repo')
import numpy as np
from ml_dtypes import bfloat16
import concourse.bass as bass
import concourse.mybir as mybir
import concourse.tile as tile
import concourse.bacc as bacc
from concourse.bass_utils import run_bass_kernel_spmd

DT = mybir.dt.float32
DTM = mybir.dt.bfloat16
B2, H, F, T_FULL = 32, 512, 512, 1024
UB = 16          # steps per hardware-loop body
N_CORES = 8


def _pack_weights(Wh, Wx, b):
    Wh = np.asarray(Wh, np.float32)
    Wx = np.asarray(Wx, np.float32)
    b = np.asarray(b, np.float32)
    # raw gate order i,f,g,o; tanh-trick pre-scale 0.5 on i and o; Wh acts on 2h
    scale_gate = np.concatenate([
        np.full(H, 0.5, np.float32), np.ones(H, np.float32),
        np.ones(H, np.float32), np.full(H, 0.5, np.float32)])
    Whs = Wh * 0.5 * scale_gate[None, :]
    Wxs = Wx * scale_gate[None, :]
    bs = b * scale_gate
    W_cat = np.concatenate([Whs, Wxs, bs[None, :]], 0)  # [1025, 4H]
    perm = []
    for q in range(4):
        for gate in (0, 2, 1, 3):  # band column order [i, g, f, o]
            perm += list(range(gate * H + 128 * q, gate * H + 128 * q + 128))
    W_pad = np.zeros((9 * 128, 4 * H), np.float32)
    W_pad[:2 * H + 1] = W_cat[:, perm]
    return W_pad.reshape(9, 128, 4 * H).astype(bfloat16)


def _unband(r):
    return np.concatenate([r[32 * q:32 * q + B2] for q in range(4)], axis=1)


def _ident_band():
    return np.eye(128, dtype=np.float32)


def _build(T, max_repeat=64):
    assert T % UB == 0 and UB % 2 == 0
    AluOp = mybir.AluOpType
    nc = bacc.Bacc("TRN2", target_bir_lowering=False, debug=False)
    x_d = nc.dram_tensor("x", [B2, T + UB, F], DTM, kind="ExternalInput")
    W_d = nc.dram_tensor("W", [9, 128, 4 * H], DTM, kind="ExternalInput")
    id_d = nc.dram_tensor("ident", [128, 128], DT, kind="ExternalInput")
    ones_d = nc.dram_tensor("ones", [1, 32], DTM, kind="ExternalInput")
    r_d = nc.dram_tensor("r", [1, 1], mybir.dt.int32, kind="ExternalInput")
    cO_d = nc.dram_tensor("c_out", [128, 128], DT, kind="ExternalOutput")
    hO_d = nc.dram_tensor("h_out", [128, 128], DT, kind="ExternalOutput")

    AF = mybir.ActivationFunctionType
    HB = UB // 8
    with tile.TileContext(nc) as tc:
        with tc.tile_pool(name="sb", bufs=1) as pool, \
             tc.tile_pool(name="ps", bufs=1, space="PSUM") as pps:
            W_s = pool.tile([128, 9 * 2048], DTM, name="W_s")
            id_s = pool.tile([128, 128], DT, name="id_s")
            ones_s = pool.tile([1, 32], DTM, name="ones_s")
            r_s = pool.tile([1, 1], mybir.dt.int32, name="r_s")
            C_s = pool.tile([128, 128], DT, name="C_s")
            hT = [pool.tile([128, 128], DTM, name=f"hT{p}") for p in range(2)]
            xst = [pool.tile([128, 1024], DTM, name=f"xst{i}") for i in range(HB)]
            th_s = [pool.tile([128, 512], DT, name=f"th{p}") for p in range(2)]
            t1_s = [pool.tile([128, 128], DT, name=f"t1{p}") for p in range(2)]
            u_s = [pool.tile([128, 128], DT, name=f"u{p}") for p in range(2)]
            tc_s = [pool.tile([128, 128], DT, name=f"tc{p}") for p in range(2)]
            aux_s = [pool.tile([128, 128], DT, name=f"aux{p}") for p in range(2)]
            ho_s = [pool.tile([128, 128], DT, name=f"ho{p}") for p in range(2)]
            h2_s = pool.tile([128, 128], DT, name="h2_s")
            gates_p = [pps.tile([128, 512], DT, name=f"g{j}") for j in range(4)]
            tpF_p = pps.tile([128, 512], DT, name="tpF")

            for k in range(9):
                nc.sync.dma_start(W_s[:, 2048 * k:2048 * (k + 1)], W_d.ap()[k])
            nc.sync.dma_start(id_s[:], id_d.ap()[:])
            nc.sync.dma_start(ones_s[:], ones_d.ap()[:])
            nc.sync.dma_start(r_s[:], r_d.ap()[:])
            R = nc.values_load(r_s[:], min_val=1, max_val=max_repeat,
                               skip_runtime_bounds_check=True)

            def xtile_k(j, k):
                i, t = divmod(j, 8)
                e = 4 * t + k
                return xst[i][:, 32 * e:32 * e + 32]

            def x_mms(j, bank):
                out = gates_p[bank]
                for g4 in range(4):
                    out_ap = out[32 * g4:32 * g4 + 32, :]
                    for n, k in enumerate([4, 5, 6, 7]):
                        nc.tensor.matmul(out_ap, xtile_k(j, k - 4),
                                         W_s[:, 2048 * k + 512 * g4:2048 * k + 512 * g4 + 512],
                                         start=(n == 0), stop=False, tile_position=(0, 32 * g4),
                                         skip_group_check=True)
                    nc.tensor.matmul(out_ap, ones_s[:],
                                     W_s[0:1, 2048 * 8 + 512 * g4:2048 * 8 + 512 * g4 + 512],
                                     start=False, stop=False, tile_position=(0, 32 * g4),
                                     skip_group_check=True)

            def h_mms(par, bank):
                out = gates_p[bank]
                for g4 in range(4):
                    out_ap = out[32 * g4:32 * g4 + 32, :]
                    for n, k in enumerate([0, 1, 2, 3]):
                        nc.tensor.matmul(out_ap, hT[par][:, 32 * k:32 * k + 32],
                                         W_s[:, 2048 * k + 512 * g4:2048 * k + 512 * g4 + 512],
                                         start=False, stop=(n == 3), tile_position=(0, 32 * g4),
                                         skip_group_check=True)

            def chain(par, bank):
                th = th_s[par]; t1 = t1_s[par]; u = u_s[par]
                tcs = tc_s[par]; aux = aux_s[par]; ho = ho_s[par]
                g = gates_p[bank]
                npar = 1 - par
                nc.scalar.activation(th[:, 0:256], g[:, 0:256], AF.Tanh)
                nc.scalar.activation(th[:, 256:384], g[:, 256:384], AF.Sigmoid)
                nc.scalar.activation(th[:, 384:512], g[:, 384:512], AF.Tanh)
                nc.vector.scalar_tensor_tensor(t1[:], th[:, 0:128], 1.0, th[:, 128:256],
                                               op0=AluOp.add, op1=AluOp.mult)
                nc.gpsimd.tensor_mul(u[:], th[:, 256:384], C_s[:])
                nc.vector.tensor_add(C_s[:], u[:], t1[:])
                nc.scalar.activation(tcs[:], C_s[:], AF.Tanh, scale=0.5)
                nc.gpsimd.tensor_scalar_add(aux[:], th[:, 384:512], 1.0)
                nc.vector.tensor_mul(ho[:], aux[:], tcs[:])
                # one full 128x128 PE transpose + one copy (vs 4+4 small ones)
                nc.tensor.transpose(tpF_p[:, 0:128], ho[:, 0:128], id_s[:, 0:128])
                nc.scalar.copy(hT[npar][:], tpF_p[:, 0:128])

            def stage_dma(i, t_expr):
                s = x_d.ap()[:, bass.ds(t_expr, 8), :].rearrange("b t f -> b (t f)")
                nc.sync.dma_start_transpose(xst[i][:].rearrange("p (e b) -> p e b", b=32), s)

            nbody = T // UB
            with tc.For_i(0, R) as rep:
                nc.vector.memset(C_s[:], 0.0)
                for p in range(2):
                    nc.vector.memset(hT[p][:], 0.0)
                for i in range(HB):
                    stage_dma(i, 8 * i)
                x_mms(0, 0)
                with tc.For_i(0, nbody) as tb:
                    next_t0 = tb * UB + UB
                    for j in range(UB):
                        sj = j % 4
                        par = j % 2
                        h_mms(par, sj)
                        x_mms((j + 1) % UB, (sj + 1) % 4)
                        if j % 8 == 7:
                            i = j // 8
                            stage_dma(i, next_t0 + 8 * i)
                        chain(par, sj)

            # outputs: c = C/2 and h = h2/2 recovered on host
            lpar = (UB - 1) % 2
            nc.vector.tensor_scalar_add(h2_s[:], th_s[lpar][:, 384:512], 1.0)
            nc.vector.tensor_mul(h2_s[:], h2_s[:], tc_s[lpar][:])
            nc.sync.dma_start(cO_d.ap()[:], C_s[:])
            nc.sync.dma_start(hO_d.ap()[:], h2_s[:])

    nc.finalize()
    return nc


_NC_CACHE = {}


def kernel(inputs, Wx, Wh, b):
    x = np.asarray(inputs, np.float32)
    Wx = np.asarray(Wx, np.float32)
    Wh = np.asarray(Wh, np.float32)
    b = np.asarray(b, np.float32)
    Bf, T, _ = x.shape
    assert Bf == 2 * B2
    if T not in _NC_CACHE:
        _NC_CACHE[T] = _build(T)
    nc = _NC_CACHE[T]

    W = _pack_weights(Wh, Wx, b)
    ident = _ident_band()
    ones = np.ones((1, 32), bfloat16)
    r1 = np.array([[1]], np.int32)
    xb = np.zeros((Bf, T + UB, x.shape[2]), bfloat16)
    xb[:, :T] = x.astype(bfloat16)
    in_maps = []
    for core in range(N_CORES):
        half = 1 if core == 1 else 0
        in_maps.append({"x": np.ascontiguousarray(xb[B2 * half:B2 * half + B2]),
                        "W": W, "ident": ident, "ones": ones, "r": r1})
    res = run_bass_kernel_spmd(nc, in_maps, list(range(N_CORES)))
    c = np.concatenate([_unband(res.results[0]["c_out"]), _unband(res.results[1]["c_out"])], 0) * 0.5
    h = np.concatenate([_unband(res.results[0]["h_out"]), _unband(res.results[1]["h_out"])], 0) * 0.5
    return c, h

